# revision 39
# baseline (speedup 1.0000x reference)
"""Fused pre-norm multi-head attention block on 8 TRN2 NeuronCores.

Sharding: data-parallel over (batch, sequence-half): core c owns batch c//2,
query rows (c%2)*1024..+1024. Zero inter-core communication: each core
computes K/V for its batch's (mask-compacted) key set locally.

Key compaction: masked keys get exactly 0 attention weight in the reference
(-inf scores), so we gather only unmasked key rows on the host (numpy), pad
to a multiple of 128 with bias -30 (exp(-30+s) ~ 1e-11, negligible), and run
dense attention over J ~= 1152 instead of 2048 keys.

Device pipeline (per core), all matmuls bf16 with f32 PSUM accumulation.
The kernel is ACT(scalar-engine)-bound on the softmax exp, so the design
keeps every other ACT op off the attention phase and maximizes exp size:

  A: LN(x) rows: DVE bn_stats; apply on ACT (per-row scale/bias); transpose.
  B: K/V/Q projections. QK-layernorm is restructured so only Q is normalized:
     qn rows are zero-mean per head, so K's mean term cancels in qn.k, and
     K's rsqrt(var+eps) folds into the exp's per-partition scale operand.
     K is evicted raw; rk is computed per (key, head) from bn_stats.
  C: per head-pair p: S = K_raw^T.T @ Q_n^T (2 heads packed in the 128-row
     PE via tile_position); one [128,1024] exp per head with scale=rk and
     bias=mask; O (V stationary, col-packed 2 heads) and den (all-ones
     stationary, replicated over 64 partitions) accumulate in PSUM; the
     softmax normalize is then a single lane-aligned DVE divide O/den
     written directly in OT layout (no broadcast needed).
  D: out = O_norm^T.T @ Wo^T -> f32 out
"""

import numpy as np
import ml_dtypes

import concourse.bacc as bacc
import concourse.bass as bass
import concourse.mybir as mybir
from concourse.tile import TileContext
from concourse.bass_utils import run_bass_kernel_spmd

BF16 = ml_dtypes.bfloat16
F32 = mybir.dt.float32
BF = mybir.dt.bfloat16
AF = mybir.ActivationFunctionType
ALU = mybir.AluOpType
AX = mybir.AxisListType

B, N, D, H, DH = 4, 2048, 1024, 16, 64
NQ = N // 2          # query rows per core
NQt = NQ // 128
EPS = 1e-5
MASK_BIAS = -30.0
LN8 = 2.0794415416798357  # 0.5*ln(64): rk = rsqrt(var+eps) = exp(-.5*ln(64v+64eps)+ln8)


def build_kernel(J: int, reps: int = 1):
    """Build the per-core Bass graph. J = padded key count (multiple of 128)."""
    assert J % 128 == 0
    Jt = J // 128
    nc = bacc.Bacc()

    xq_d = nc.declare_dram_parameter("xq", [NQ, D], F32, isOutput=False)
    xkv_d = nc.declare_dram_parameter("xkv", [J, D], F32, isOutput=False)
    wqT_d = nc.declare_dram_parameter("wqT", [D, D], BF, isOutput=False)
    wkT_d = nc.declare_dram_parameter("wkT", [D, D], BF, isOutput=False)
    wvT_d = nc.declare_dram_parameter("wvT", [D, D], BF, isOutput=False)
    woT_d = nc.declare_dram_parameter("woT", [D, D], BF, isOutput=False)
    biasT_d = nc.declare_dram_parameter("biasT", [128, Jt], F32, isOutput=False)
    out_d = nc.declare_dram_parameter("out", [NQ, D], F32, isOutput=True)

    from contextlib import ExitStack

    with TileContext(nc) as tc:
        loop_ctx = tc.For_i(0, reps) if reps > 1 else None
        if loop_ctx is not None:
            loop_ctx.__enter__()
        try:
            with ExitStack() as ctx:
                _body(ctx, tc, nc, J, Jt,
                      xq_d, xkv_d, wqT_d, wkT_d, wvT_d, woT_d, biasT_d, out_d)
        finally:
            if loop_ctx is not None:
                loop_ctx.__exit__(None, None, None)
    nc.finalize()
    return nc


I32 = mybir.dt.int32
RSQRT_MAGIC = 0x5F3759DF


def _grouped_stats(nc, pool, pt, sq, eps64t):
    """Per-head (sums, M2+64eps) of a [128,1024] projection PSUM tile.

    pt: PSUM f32 [128, 1024] (16 head groups of 64); sq: SBUF bf16 squares
    of pt (computed on ACT Square — same table set as Exp, no reload).
    Returns (sums, v=M2+64eps) [128, 16] f32 tiles.
    """
    g = pt[:].rearrange("p (g s) -> p g s", s=DH)
    gsq = sq[:].rearrange("p (g s) -> p g s", s=DH)
    sums = pool.tile([128, 16], F32, tag="sums")
    sqs = pool.tile([128, 16], F32, tag="sqs")
    nc.vector.reduce_sum(out=sums[:], in_=g, axis=AX.X)
    nc.vector.reduce_sum(out=sqs[:], in_=gsq, axis=AX.X)
    s2 = pool.tile([128, 16], F32, tag="s2")
    nc.vector.tensor_mul(out=s2[:], in0=sums[:], in1=sums[:])
    v = pool.tile([128, 16], F32, tag="v")
    nc.vector.scalar_tensor_tensor(out=v[:], in0=s2[:], scalar=-1.0 / DH,
                                   in1=sqs[:], op0=ALU.mult, op1=ALU.add)
    nc.vector.tensor_tensor(out=v[:], in0=v[:], in1=eps64t, op=ALU.add)
    return sums, v


def _rsqrt(nc, pool, v, magic, out, c=1.0, iters=2):
    """out = c * v^-0.5 on DVE: bit-trick seed + Newton iterations.

    v: [128, G] f32 AP (positive, normal range); magic: [128, G] int32 AP
    pre-set to 0x5f3759df; out: [128, G] f32 AP. iters=2 -> ~5e-6 rel err;
    iters=1 -> ~2e-3 (fine for bf16 outputs).
    """
    G = v.shape[1]
    h = pool.tile([128, G], I32, tag="rs_h")
    nc.vector.tensor_scalar(out=h[:], in0=v.bitcast(I32), scalar1=1,
                            scalar2=None, op0=ALU.logical_shift_right)
    nc.vector.tensor_sub(out=h[:], in0=magic, in1=h[:])
    y = h[:].bitcast(F32)
    a = pool.tile([128, G], F32, tag="rs_a")
    u = pool.tile([128, G], F32, tag="rs_u")
    for it in range(iters):
        last = it == iters - 1
        nc.vector.tensor_mul(out=a[:], in0=y, in1=y)
        nc.vector.tensor_mul(out=a[:], in0=a[:], in1=v)
        nc.vector.tensor_scalar(out=u[:], in0=a[:],
                                scalar1=-0.5 * (c if last else 1.0),
                                scalar2=1.5 * (c if last else 1.0),
                                op0=ALU.mult, op1=ALU.add)
        if last:
            nc.vector.tensor_mul(out=out, in0=y, in1=u[:])
        else:
            y1 = pool.tile([128, G], F32, tag="rs_y1")
            nc.vector.tensor_mul(out=y1[:], in0=y, in1=u[:])
            y = y1[:]


def _body(ctx, tc, nc, J, Jt, xq_d, xkv_d, wqT_d, wkT_d, wvT_d, woT_d,
          biasT_d, out_d):
    from contextlib import ExitStack

    # ---- long-lived SBUF tensors ----
    statics = ctx.enter_context(tc.tile_pool(name="statics", bufs=1))
    # transposed tensors are source-tile-major [128, ntile, 8, 128] so each
    # dma_start_transpose writes one contiguous [128, 8, 128] block
    xqT = statics.tile([128, NQt, 8, 128], BF, tag="xqT")      # xn_q^T
    xkvT = statics.tile([128, Jt, 8, 128], BF, tag="xkvT")     # xn_kv^T
    QT = statics.tile([128, NQt, 8, 128], BF, tag="QT")        # Q_ln^T
    KT = statics.tile([128, Jt, 8, 128], BF, tag="KT")         # K_raw^T
    Vsb = statics.tile([128, Jt, H, DH], BF, tag="Vsb")        # V natural
    OT = statics.tile([128, NQt, 8, 128], BF, tag="OT")        # O_norm^T
    bias_sb = statics.tile([128, Jt], F32, tag="bias")
    rk_sb = statics.tile([128, Jt, H], F32, tag="rk")          # rsqrt(var_k+eps)
    ones64 = statics.tile([128, 64], BF, tag="ones64")
    eps1 = statics.tile([128, 1], F32, tag="eps1")
    eps64 = statics.tile([128, 16], F32, tag="eps64")
    magic = statics.tile([128, 16], I32, tag="magic")
    nc.sync.dma_start(out=bias_sb[:], in_=biasT_d[:])
    nc.vector.memset(eps1[:], EPS)
    nc.vector.memset(eps64[:], float(DH * EPS))
    nc.vector.memset(ones64[:], 1.0)
    nc.vector.memset(magic[:], RSQRT_MAGIC)

    # ---- weights: trigger DMAs first so transfers overlap phase A ----
    bctx = ExitStack()
    wpool = bctx.enter_context(tc.tile_pool(name="wpool", bufs=1))
    wq_sb = wpool.tile([128, 8, D], BF, tag="wq")
    wk_sb = wpool.tile([128, 8, D], BF, tag="wk")
    wv_sb = wpool.tile([128, 8, D], BF, tag="wv")
    # weights are pre-shuffled on the host to [p, t, e] row order, so these
    # DMAs are fully contiguous (1 descriptor per partition)
    nc.scalar.dma_start(out=wq_sb[:], in_=wqT_d[:].rearrange("(p t) e -> p t e", p=128))
    nc.scalar.dma_start(out=wk_sb[:], in_=wkT_d[:].rearrange("(p t) e -> p t e", p=128))
    nc.scalar.dma_start(out=wv_sb[:], in_=wvT_d[:].rearrange("(p t) e -> p t e", p=128))

    # ---- phase A: layernorm x rows + transpose ----
    with ExitStack() as actx:
        xpool = actx.enter_context(tc.tile_pool(name="xpool", bufs=4))
        xnpool = actx.enter_context(tc.tile_pool(name="xnpool", bufs=3))
        aspool = actx.enter_context(tc.tile_pool(name="aspool", bufs=6))

        def ln_rows(src_d, ntiles, dstT, tg):
            xn = None
            for nt in range(ntiles):
                xt = xpool.tile([128, D], F32, tag="xt")
                nc.sync.dma_start(out=xt[:], in_=src_d[nt * 128:(nt + 1) * 128, :])
                st = aspool.tile([128, 2, 6], F32, tag="st")
                mv = aspool.tile([128, 2], F32, tag="mv")
                rr = aspool.tile([128, 1], F32, tag="rr")
                nc.vector.bn_stats(out=st[:, 0, :], in_=xt[:, 0:512])
                nc.vector.bn_stats(out=st[:, 1, :], in_=xt[:, 512:1024])
                nc.vector.bn_aggr(out=mv[:], in_=st[:])
                # r = (var+eps)^-0.5 on DVE (keeps ACT free of Ln/Exp table
                # thrash; the exp table then loads exactly once)
                ve = aspool.tile([128, 1], F32, tag="ve")
                nc.vector.tensor_tensor(out=ve[:], in0=mv[:, 1:2], in1=eps1[:],
                                        op=ALU.add)
                _rsqrt(nc, aspool, ve[:], magic[:, 0:1], rr[:], iters=1)
                nmr = aspool.tile([128, 1], F32, tag="nmr")
                nc.vector.tensor_scalar(out=nmr[:], in0=mv[:, 0:1],
                                        scalar1=-1.0, scalar2=rr[:],
                                        op0=ALU.mult, op1=ALU.mult)
                # group tg LN'd tiles per transpose (saves the ~800ns fixed
                # trigger cost per dma_start_transpose)
                if nt % tg == 0:
                    g = min(tg, ntiles - nt)
                    xn = xnpool.tile([128, g, D], BF, tag="xn")
                nc.scalar.activation(out=xn[:, nt % tg, :], in_=xt[:], func=AF.Identity,
                                     bias=nmr[:], scale=rr[:])
                if nt % tg == g - 1:
                    nc.sync.dma_start_transpose(
                        out=dstT[:, nt - g + 1:nt + 1, :, :],
                        in_=xn[:].rearrange("p g e -> p (g e)"))

        ln_rows(xkv_d, Jt, xkvT, 3)
        ln_rows(xq_d, NQt, xqT, 4)

    # ---- phase B ---- K and V projections ---

    with ExitStack() as kctx:
        psK = kctx.enter_context(tc.tile_pool(name="psK", bufs=2, space="PSUM"))
        psV = kctx.enter_context(tc.tile_pool(name="psV", bufs=2, space="PSUM"))
        kpool = kctx.enter_context(tc.tile_pool(name="kpool", bufs=3))
        kstat = kctx.enter_context(tc.tile_pool(name="kstat", bufs=8))
        for nt in range(Jt):
            kp = psK.tile([128, 1024], F32, tag="kp")
            vp = psV.tile([128, 1024], F32, tag="vp")
            for dt in range(8):
                lhs = xkvT[:, nt, dt, :]
                nc.tensor.matmul(kp[:, 0:512], lhs, wk_sb[:, dt, 0:512],
                                 start=(dt == 0), stop=(dt == 7))
                nc.tensor.matmul(kp[:, 512:1024], lhs, wk_sb[:, dt, 512:1024],
                                 start=(dt == 0), stop=(dt == 7), skip_group_check=True)
                nc.tensor.matmul(vp[:, 0:512], lhs, wv_sb[:, dt, 0:512],
                                 start=(dt == 0), stop=(dt == 7), skip_group_check=True)
                nc.tensor.matmul(vp[:, 512:1024], lhs, wv_sb[:, dt, 512:1024],
                                 start=(dt == 0), stop=(dt == 7), skip_group_check=True)
            # per (key, head) rk = rsqrt(var+eps) -> rk_sb; mean is not needed
            # (qn rows are zero-mean so K's mean cancels in qn.k).
            # rk = 8*rsqrt(M2+64eps), folded into the last Newton step.
            ksq = kstat.tile([128, 1024], BF, tag="ksq")
            nc.scalar.square(out=ksq[:], in_=kp[:])
            _, v = _grouped_stats(nc, kstat, kp, ksq, eps64[:])
            _rsqrt(nc, kstat, v[:], magic[:], rk_sb[:, nt, :], c=8.0)
            if nt % 3 == 0:
                kg = min(3, Jt - nt)
                kn = kpool.tile([128, kg, 1024], BF, tag="kn")
            nc.scalar.activation(out=kn[:, nt % 3, :], in_=kp[:], func=AF.Copy)
            if nt % 3 == kg - 1:
                nc.sync.dma_start_transpose(
                    out=KT[:, nt - kg + 1:nt + 1, :, :],
                    in_=kn[:].rearrange("p g e -> p (g e)"))
            nc.scalar.activation(out=Vsb[:, nt, :, :], in_=vp[:].rearrange(
                "p (h s) -> p h s", s=DH), func=AF.Copy)

    # --- Q projection + full per-head LN (scale folds DH^-0.5) ---
    with ExitStack() as qctx:
        psQ = qctx.enter_context(tc.tile_pool(name="psQ", bufs=2, space="PSUM"))
        qpool = qctx.enter_context(tc.tile_pool(name="qpool", bufs=3))
        qstat = qctx.enter_context(tc.tile_pool(name="qstat", bufs=8))
        for nt in range(NQt):
            qp = psQ.tile([128, 1024], F32, tag="qp")
            for dt in range(8):
                lhs = xqT[:, nt, dt, :]
                nc.tensor.matmul(qp[:, 0:512], lhs, wq_sb[:, dt, 0:512],
                                 start=(dt == 0), stop=(dt == 7))
                nc.tensor.matmul(qp[:, 512:1024], lhs, wq_sb[:, dt, 512:1024],
                                 start=(dt == 0), stop=(dt == 7), skip_group_check=True)
            qsq = qstat.tile([128, 1024], BF, tag="qsq")
            nc.scalar.square(out=qsq[:], in_=qp[:])
            sums, v = _grouped_stats(nc, qstat, qp, qsq, eps64[:])
            m = qstat.tile([128, 16], F32, tag="m")
            nc.vector.tensor_scalar_mul(out=m[:], in0=sums[:], scalar1=1.0 / DH)
            # rq = rsqrt(64*(var+eps)) = rsqrt(M2+64eps): folds DH^-0.5
            rq = qstat.tile([128, 16], F32, tag="rq")
            _rsqrt(nc, qstat, v[:], magic[:], rq[:])
            # apply (q-m)*rq per head group; ACT needs bias=-m*rq precomputed
            nmr = qstat.tile([128, 16], F32, tag="nmr")
            nc.vector.scalar_tensor_tensor(out=nmr[:], in0=m[:], scalar=-1.0,
                                           in1=rq[:], op0=ALU.mult, op1=ALU.mult)
            if nt % 4 == 0:
                qg = min(4, NQt - nt)
                qn = qpool.tile([128, qg, 1024], BF, tag="qn")
            for g in range(16):
                gs = slice(g * 64, (g + 1) * 64)
                nc.vector.tensor_scalar(
                    out=qn[:, nt % 4, gs], in0=qp[:, gs],
                    scalar1=m[:, g:g + 1], scalar2=rq[:, g:g + 1],
                    op0=ALU.subtract, op1=ALU.mult)
            if nt % 4 == qg - 1:
                nc.sync.dma_start_transpose(
                    out=QT[:, nt - qg + 1:nt + 1, :, :],
                    in_=qn[:].rearrange("p g e -> p (g e)"))

    bctx.close()  # free Wq/Wk/Wv SBUF before attention

    # ---- phase C: attention per head pair ----
    wopool = ctx.enter_context(tc.tile_pool(name="wopool", bufs=1))
    wo_sb = wopool.tile([128, 8, D], BF, tag="wo")
    nc.sync.dma_start(out=wo_sb[:], in_=woT_d[:].rearrange("(p t) e -> p t e", p=128))

    cctx = ExitStack()
    epool = cctx.enter_context(tc.tile_pool(name="epool", bufs=4))
    dpool = cctx.enter_context(tc.tile_pool(name="dpool", bufs=2))
    psS = cctx.enter_context(tc.tile_pool(name="psS", bufs=2, space="PSUM"))
    psO = cctx.enter_context(tc.tile_pool(name="psO", bufs=1, space="PSUM"))
    psDen = cctx.enter_context(tc.tile_pool(name="psDen", bufs=1, space="PSUM"))

    def qk(p, jt):
        Sa = psS.tile([128, 1024], F32, tag="S")
        Sb = psS.tile([128, 1024], F32, tag="S")
        nc.tensor.matmul(Sa[:, 0:512], KT[0:64, jt, p, :], QT[0:64, 0:4, p, :],
                         start=True, stop=True, tile_position=(0, 0))
        nc.tensor.matmul(Sa[:, 512:1024], KT[0:64, jt, p, :], QT[0:64, 4:8, p, :],
                         start=True, stop=True, tile_position=(0, 0),
                         skip_group_check=True)
        nc.tensor.matmul(Sb[:, 0:512], KT[64:128, jt, p, :], QT[64:128, 0:4, p, :],
                         start=True, stop=True, tile_position=(64, 0),
                         skip_group_check=True)
        nc.tensor.matmul(Sb[:, 512:1024], KT[64:128, jt, p, :], QT[64:128, 4:8, p, :],
                         start=True, stop=True, tile_position=(64, 0),
                         skip_group_check=True)
        return Sa, Sb

    for p in range(8):
        ha, hb = 2 * p, 2 * p + 1
        O = psO.tile([128, 1024], F32, tag="O")
        den = psDen.tile([128, 1024], F32, tag="den")
        Sa, Sb = qk(p, 0)
        for jt in range(Jt):
            first, last = (jt == 0), (jt == Jt - 1)
            Ea = epool.tile([128, 1024], BF, tag="Ea")
            Eb = epool.tile([128, 1024], BF, tag="Eb")
            nc.scalar.activation(out=Ea[:], in_=Sa[:], func=AF.Exp,
                                 bias=bias_sb[:, jt:jt + 1],
                                 scale=rk_sb[:, jt, ha:ha + 1])
            nc.scalar.activation(out=Eb[:], in_=Sb[:], func=AF.Exp,
                                 bias=bias_sb[:, jt:jt + 1],
                                 scale=rk_sb[:, jt, hb:hb + 1])
            # head a: AV + den, then next tile's QK(a) so ACT never waits
            for c in range(2):
                cs = slice(c * 512, (c + 1) * 512)
                nc.tensor.matmul(O[0:64, cs], Vsb[:, jt, ha, :], Ea[:, cs],
                                 start=first, stop=last, tile_position=(0, 0),
                                 skip_group_check=True)
                nc.tensor.matmul(den[0:64, cs], ones64[:], Ea[:, cs],
                                 start=first, stop=last, tile_position=(0, 0),
                                 skip_group_check=True)
            if not last:
                Sa2, Sb2 = qk(p, jt + 1)
            for c in range(2):
                cs = slice(c * 512, (c + 1) * 512)
                nc.tensor.matmul(O[64:128, cs], Vsb[:, jt, hb, :], Eb[:, cs],
                                 start=first, stop=last, tile_position=(0, 64),
                                 skip_group_check=True)
                nc.tensor.matmul(den[64:128, cs], ones64[:], Eb[:, cs],
                                 start=first, stop=last, tile_position=(0, 64),
                                 skip_group_check=True)
            if not last:
                Sa, Sb = Sa2, Sb2
        # softmax normalize: O/den, lane-aligned (den replicated per head
        # half by the all-ones stationary), written directly in OT layout
        dsb = dpool.tile([128, 1024], F32, tag="dsb")
        nc.vector.reciprocal(out=dsb[:], in_=den[:])
        nc.vector.tensor_tensor(
            out=OT[:, :, p, :],
            in0=O[:].rearrange("p (a b) -> p a b", b=128),
            in1=dsb[:].rearrange("p (a b) -> p a b", b=128),
            op=ALU.mult)

    cctx.close()  # free attention PSUM/SBUF pools

    # ---- phase D: output projection ----
    psD = ctx.enter_context(tc.tile_pool(name="psD", bufs=2, space="PSUM"))
    opool = ctx.enter_context(tc.tile_pool(name="opool", bufs=3))
    for it in range(NQt):
        po = psD.tile([128, 1024], F32, tag="po")
        for ft in range(8):
            lhs = OT[:, it, ft, :]
            nc.tensor.matmul(po[:, 0:512], lhs, wo_sb[:, ft, 0:512],
                             start=(ft == 0), stop=(ft == 7))
            nc.tensor.matmul(po[:, 512:1024], lhs, wo_sb[:, ft, 512:1024],
                             start=(ft == 0), stop=(ft == 7), skip_group_check=True)
        ost = opool.tile([128, 1024], F32, tag="ost")
        nc.scalar.activation(out=ost[:], in_=po[:], func=AF.Copy)
        nc.gpsimd.dma_start(out=out_d[it * 128:(it + 1) * 128, :], in_=ost[:])


def prepare_inputs(x, attention_mask, norm_w, norm_b, qn_w, qn_b, kn_w, kn_b,
                   Wq, bq, Wk, bk, Wv, bv, Wo):
    """Host-side sharding/folding. Returns (J, in_maps for cores 0..7)."""
    x = np.asarray(x, np.float32)
    mask = np.asarray(attention_mask)
    for nm, a in (("norm_b", norm_b), ("bq", bq), ("bk", bk), ("bv", bv),
                  ("qn_b", qn_b), ("kn_b", kn_b)):
        assert np.abs(np.asarray(a)).max() == 0.0, f"{nm} != 0 unsupported"
    for nm, a in (("qn_w", qn_w), ("kn_w", kn_w)):
        assert np.abs(np.asarray(a) - 1.0).max() == 0.0, f"{nm} != 1 unsupported"
    norm_w = np.asarray(norm_w, np.float32)

    counts = mask.sum(1)
    J = int(max(128, np.ceil(counts.max() / 128) * 128))
    Jt = J // 128

    def shuffle(w):
        # [t*128+p, e] -> [p*8+t, e] so the device DMA to SBUF layout
        # [p, t, e] reads contiguous rows (1 descriptor per partition)
        w = np.ascontiguousarray(w).astype(BF16)
        return np.ascontiguousarray(w.reshape(8, 128, D).transpose(1, 0, 2).reshape(D, D))

    wqT = shuffle((np.asarray(Wq) * norm_w[None, :]).T)
    wkT = shuffle((np.asarray(Wk) * norm_w[None, :]).T)
    wvT = shuffle((np.asarray(Wv) * norm_w[None, :]).T)
    woT = shuffle(np.asarray(Wo).T)

    in_maps = []
    for c in range(8):
        b, half = c // 2, c % 2
        idx = np.flatnonzero(mask[b])
        pad = J - len(idx)
        idxp = np.concatenate([idx, np.zeros(pad, np.int64)])
        bias = np.concatenate([np.zeros(len(idx), np.float32),
                               np.full(pad, MASK_BIAS, np.float32)])
        biasT = np.ascontiguousarray(bias.reshape(Jt, 128).T)
        in_maps.append({
            "xq": np.ascontiguousarray(x[b, half * NQ:(half + 1) * NQ]),
            "xkv": np.ascontiguousarray(x[b][idxp]),
            "wqT": wqT, "wkT": wkT, "wvT": wvT, "woT": woT,
            "biasT": biasT,
        })
    return J, in_maps


_CACHE = {}


def kernel(**inputs) -> np.ndarray:
    J, in_maps = prepare_inputs(**inputs)
    key = (J, 1)
    if key not in _CACHE:
        _CACHE[key] = build_kernel(J, reps=1)
    nc = _CACHE[key]
    res = run_bass_kernel_spmd(nc, in_maps, list(range(8)))
    out = np.empty((B, N, D), np.float32)
    for c in range(8):
        b, half = c // 2, c % 2
        out[b, half * NQ:(half + 1) * NQ] = res.results[c]["out"]
    return out
